# revision 28
# baseline (speedup 1.0000x reference)
"""Multi-head attention (B=2, T=2048, E=1024, H=16) on 8 TRN2 NeuronCores.

Sharding: core c handles batch c//4 and head group c%4 (4 heads of 64 dims
-> 256 columns of w_Q/w_K/w_V and of the output). Pure SPMD, no collectives:
every core runs the same NEFF on its own input shard.

Per-core kernel (all matmul operands bf16, PSUM/softmax math fp32):
  xT [E, T] (host pre-transposed), wq/wk/wv [E, 256]
  1. QT/KT per head-pair p: [128, T] = (w pair-slice)^T @ xT   (PE)
  2. V per s-tile: [128, 4*65] with a ones column per head     (PE + DVE copy)
  3. scores transposed per head: ST[s, t] = K Q^T, two heads packed into
     PE row groups (K=64 each) writing one [128, 1024] PSUM tile
  4. exp via ACT straight from PSUM, scale=1/8 folded into the activation
     affine, bf16 out -> PT
  5. attn: out[t,65] = PT_slice^T @ V_aug accumulated over 16 s-chunks;
     col 64 = softmax denominator (from the ones column)
  6. normalize: DVE reciprocal + per-partition tensor_scalar mul -> fp32 out
"""

import numpy as np
import ml_dtypes

B, T, E, H = 2, 2048, 1024, 16
D = 64          # head dim
HG = 4          # heads per core
GC = HG * D     # 256 output columns per core
NCORES = 8

_cached_nc = None


def _build_program(seq: int = T, reps: int = 1, skip_attn=False, skip_exp=False, _nosplit=False):
    """reps>1 emits the body multiple times in one NEFF (timing only).
    skip_attn/skip_exp build ablation variants for HW phase attribution."""
    import concourse.bacc as bacc
    import concourse.tile as tile
    from concourse import mybir

    bf16 = mybir.dt.bfloat16
    f32 = mybir.dt.float32
    i16 = mybir.dt.int16
    Exp = mybir.ActivationFunctionType.Exp
    Mult = mybir.AluOpType.mult
    Add = mybir.AluOpType.add
    # Schraudolph fast-exp constants (2^x bit trick), with the 1/sqrt(HD)
    # score scale folded into the multiplier like the ACT path's scale=.
    # Scaled by 2^-16 so the rounded result IS the bf16 bit pattern as
    # int16 — one DVE op, no separate bitcast-narrow pass.
    SCH_C1 = (1 << 7) * 1.4426950408889634 * 0.125
    SCH_C2 = (1 << 7) * (127.0 - 0.04367744)
    SCH_TAIL = 11  # exp groups >= this index (per unit) go to DVE

    NT = seq // 128     # s-tiles / t-tiles
    NTC = seq // 512    # 512-wide t-chunks
    KO = E // 128       # contraction chunks for projections

    nc = bacc.Bacc(
        "TRN2", target_bir_lowering=False, debug=False, num_devices=NCORES
    )

    xT_d = nc.dram_tensor("xT", [E, seq], bf16, kind="ExternalInput")
    wq_d = nc.dram_tensor("wq", [E, GC], bf16, kind="ExternalInput")
    wk_d = nc.dram_tensor("wk", [E, GC], bf16, kind="ExternalInput")
    wv_d = nc.dram_tensor("wv", [E, GC], bf16, kind="ExternalInput")
    out_d = nc.dram_tensor("out", [seq, GC], f32, kind="ExternalOutput")

    with tile.TileContext(nc) as tc:
        with (
            tc.tile_pool(name="singles", bufs=1) as singles,
            tc.tile_pool(name="pt", bufs=3) as ptp,
            tc.tile_pool(name="stage", bufs=8) as stagep,
            tc.tile_pool(name="recip", bufs=8) as recipp,
            # PSUM budget (8 banks): scores 3x[128,1024] (6) + attn 1 + proj 1
            tc.tile_pool(name="proj_ps", bufs=1, space="PSUM") as proj_ps,
            tc.tile_pool(name="score_ps", bufs=3, space="PSUM") as score_ps,
            tc.tile_pool(name="attn_ps", bufs=1, space="PSUM") as attn_ps,
        ):
          for _rep in range(reps):
            # ---- load inputs, ramp-ordered: the first projection chains
            # need only the pair-0 weight halves + xT t-chunk 0, so those DMA
            # first and the rest follows behind.
            wq = singles.tile([128, KO, GC], bf16)
            wk = singles.tile([128, KO, GC], bf16)
            wv = singles.tile([128, KO, GC], bf16)
            xT = singles.tile([128, KO, seq], bf16)

            # Weights ride the gpsimd DMA ring so their transfers overlap
            # the xT stream on the sync ring instead of queueing behind it.
            def dma_w(dst, src, p):
                cols = slice(p * 128, (p + 1) * 128)
                nc.gpsimd.dma_start(
                    dst[:, :, cols],
                    src[:, cols].rearrange("(ko p) c -> p ko c", p=128),
                )

            def dma_x(tcq):
                for k in range(KO):
                    nc.sync.dma_start(
                        xT[:, k, tcq * 512:(tcq + 1) * 512],
                        xT_d[k * 128:(k + 1) * 128, tcq * 512:(tcq + 1) * 512],
                    )

            dma_w(wq, wq_d, 0)
            dma_w(wk, wk_d, 0)
            dma_x(0)
            dma_w(wq, wq_d, 1)
            dma_w(wk, wk_d, 1)
            for tcq in range(1, NTC):
                dma_x(tcq)
            nc.gpsimd.dma_start(
                wv[:], wv_d[:].rearrange("(ko p) c -> p ko c", p=128))

            # QT/KT: [128, pair, seq]; partitions 0-63 head 2p, 64-127 head 2p+1
            QT = singles.tile([128, 2, seq], bf16)
            KT = singles.tile([128, 2, seq], bf16)
            # V with ones col per head: [128, s-tile, 4*65]; only the ones
            # columns need the memset — proj_v overwrites the data columns.
            V = singles.tile([128, NT, HG * (D + 1)], bf16)
            nc.vector.memset(
                V[:].rearrange("p st (h c) -> p st h c", h=HG)[:, :, :, D:D + 1],
                1.0,
            )

            def proj_qk(p, w_sb, dst, tcs):
                """Project t-chunks `tcs` of QT or KT for head-pair p."""
                for tcq in tcs:
                    ps = proj_ps.tile([128, 512], f32, tag="proj")
                    for k in range(KO):
                        nc.tensor.matmul(
                            ps[:],
                            lhsT=w_sb[:, k, p * 128:(p + 1) * 128],
                            rhs=xT[:, k, tcq * 512:(tcq + 1) * 512],
                            start=(k == 0),
                            stop=(k == KO - 1),
                        )
                    nc.vector.tensor_copy(
                        out=dst[:, p, tcq * 512:(tcq + 1) * 512], in_=ps[:]
                    )

            def proj_v(tiles):
                for st in tiles:
                    ps = proj_ps.tile([128, 512], f32, tag="proj")
                    for k in range(KO):
                        nc.tensor.matmul(
                            ps[:, :GC],
                            lhsT=xT[:, k, st * 128:(st + 1) * 128],
                            rhs=wv[:, k, :],
                            start=(k == 0),
                            stop=(k == KO - 1),
                        )
                    nc.vector.tensor_copy(
                        out=V[:, st].rearrange("p (h c) -> p h c", h=HG)[:, :, :D],
                        in_=ps[:, :GC].rearrange("p (h c) -> p h c", h=HG),
                    )

            def scores_unit(p, t0, hooks=None, n_sch=None, tail=False,
                            w=512):
                """ST = K Q^T (both heads row-packed) + exp -> PT tile.

                PT layout is flat [128, NT*1024]: one s-tile's two 512-wide
                bank writes form one 2-bank exp group, so attn can chase exp
                output at s-tile granularity. n_sch groups take the DVE
                Schraudolph path: a single tensor_scalar whose rounded int16
                result is the bf16 bit pattern of 2^x (ACT alone can't keep
                up with PE in units that carry no projection filler). The
                last unit alternates engines per group so its exps drain at
                the combined ACT+DVE rate. hooks[st] emits filler just
                before s-tile st."""
                # constant tile shapes regardless of w — half-width units
                # just use the leading columns
                pt = ptp.tile([128, NT * 1024], bf16, tag="pt")
                if n_sch is None:
                    n_sch = NT - SCH_TAIL
                # Spread DVE groups evenly through the unit (clustering them
                # at the tail leaves DVE idle while ACT backlogs, and the
                # score PSUM ring is only 3 groups deep).
                sch_set = {st for st in range(NT)
                           if ((st + 1) * n_sch) // NT > (st * n_sch) // NT}
                for st in range(NT):
                    for f in (hooks or {}).get(st, []):
                        f()
                    sc = score_ps.tile([128, 1024], f32, tag="score")
                    for h in range(2):
                        # heads stay bank-strided (h*512, not h*w): the two
                        # matmuls run concurrently on different PE row
                        # groups, and concurrent writes into one single-port
                        # PSUM bank are a hardware fault.
                        nc.tensor.matmul(
                            sc[:, h * 512:h * 512 + w],
                            lhsT=KT[h * 64:(h + 1) * 64, p,
                                    st * 128:(st + 1) * 128],
                            rhs=QT[h * 64:(h + 1) * 64, p, t0:t0 + w],
                            start=True,
                            stop=True,
                        )
                    if skip_exp:
                        continue
                    dst = pt[:, st * 2 * w:(st + 1) * 2 * w]
                    src = (sc[:] if w == 512 else
                           sc[:].rearrange("p (b c) -> p b c", b=2)[:, :, :w])
                    if (st % 2 == 1) if tail else (st in sch_set):
                        nc.vector.tensor_scalar(
                            dst.bitcast(i16), src, SCH_C1, SCH_C2,
                            Mult, Add,
                        )
                    else:
                        nc.scalar.activation(
                            out=dst, in_=src, func=Exp, scale=0.125,
                        )
                return pt

            def attn_unit(p, t0, pt, w=512):
                """attn = PT^T @ V_aug accumulated over s, then normalize.
                tt-major: both heads' accumulators for one t-tile share a
                PSUM bank (each chain is a closed start..stop group), so
                each t-tile normalizes and DMAs out while the next t-tile's
                chains run — the unit's output drains incrementally instead
                of all at the end."""
                ap2 = attn_ps.tile([128, 2, 2 * (D + 1)], f32, tag="attn")
                for tt in range(w // 128):
                    # alternate halves of the bank: t-tile tt+2's first
                    # (start=True) write only has to wait for tt's normalize
                    # reads, two chains back — not the immediately preceding
                    # ones.
                    ap = ap2[:, tt % 2]
                    stg = stagep.tile([128, 128], f32, tag="stage")
                    for h in range(2):
                        hh = p * 2 + h
                        for st in range(NT):
                            nc.tensor.matmul(
                                ap[:, h * (D + 1):(h + 1) * (D + 1)],
                                lhsT=pt[:, st * 2 * w + h * w + tt * 128:
                                        st * 2 * w + h * w + (tt + 1) * 128],
                                rhs=V[:, st, hh * (D + 1):(hh + 1) * (D + 1)],
                                start=(st == 0),
                                stop=(st == NT - 1),
                            )
                    for h in range(2):
                        r = recipp.tile([128, 1], f32, tag="recip")
                        nc.vector.reciprocal(
                            out=r[:],
                            in_=ap[:, h * (D + 1) + D:h * (D + 1) + D + 1],
                        )
                        nc.vector.tensor_scalar(
                            stg[:, h * D:(h + 1) * D],
                            ap[:, h * (D + 1):h * (D + 1) + D],
                            r[:],
                            None,
                            Mult,
                        )
                    nc.sync.dma_start(
                        out_d[t0 + tt * 128:t0 + (tt + 1) * 128,
                              p * 128:(p + 1) * 128],
                        stg[:],
                    )

            # Program order is semantic order under Tile (WAR/RAW follow it),
            # and it is also the scheduler's priority order. Software-pipeline
            # the softmax: emit scores(u+1) before attn(u) so ACT never
            # starves at a unit boundary; slot filler work (V projection,
            # pair-1 QK, deferred QT-0 chunks) right after the scores that
            # precede its first use.
            # Minimal critical path to the first exp: QT0[tc0], KT0[tc0],
            # then unit-0 scores. All remaining projection work (KT0 tails,
            # QT0 tails, V, pair-1 QK) is spread through the score s-loops
            # as hook filler so PE keeps ACT fed instead of lumping
            # projections between units. attn runs two units behind scores
            # (pt pool bufs >= 3). Everything is emitted before its first
            # program-order use (Tile semantics follow program order).
            proj_qk(0, wq, QT, [0])
            proj_qk(0, wk, KT, [0])
            units = [(p, tcq * 512, 512)
                     for p in range(2) for tcq in range(NTC)]
            qk0 = lambda w, d, tcs: (lambda: proj_qk(0, w, d, tcs))
            qk1 = lambda w, d, tcs: (lambda: proj_qk(1, w, d, tcs))
            pv = lambda ts: (lambda: proj_v(ts))
            if NTC == 4:
                # Each chunk is hooked 2-4 s-tiles before its first use so
                # the PSUM->SBUF copy lands before the dependent ld/mm
                # instead of just-in-time (the copy latency otherwise stalls
                # the score pipeline at every chunk boundary).
                hooks = {
                    0: {2: [qk0(wk, KT, [1])], 5: [qk0(wk, KT, [2])],
                        8: [qk0(wk, KT, [3])], 12: [qk0(wq, QT, [1])]},
                    1: {2: [pv(range(0, 4))], 6: [qk0(wq, QT, [2])],
                        10: [pv(range(4, 8))], 14: [qk0(wq, QT, [3])]},
                    2: {2: [pv(range(8, 12))], 5: [qk1(wk, KT, [0])],
                        8: [pv(range(12, 16))], 11: [qk1(wk, KT, [1])],
                        14: [qk1(wk, KT, [2])]},
                    3: {2: [qk1(wk, KT, [3])], 6: [qk1(wq, QT, [0])],
                        10: [qk1(wq, QT, [1])], 14: [qk1(wq, QT, [2])]},
                    4: {2: [qk1(wq, QT, [3])]},
                }
                fillers = {}
            else:
                hooks = {0: {4 * c: [qk0(wk, KT, [c])] for c in range(1, NTC)}}
                fillers = {0: [qk0(wq, QT, range(1, NTC)),
                               pv(range(NT))]}
                fillers.setdefault(min(1, NTC - 1), []).extend((
                    qk1(wk, KT, range(NTC)),))
                fillers.setdefault(min(2, NTC - 1), []).append(
                    qk1(wq, QT, range(NTC)))
            # The final unit is split into two 256-wide halves so the last
            # exp batch and attn chains are half-sized — the pipeline drain
            # after the last score matmul shortens accordingly.
            if NTC == 4 and not _nosplit:
                lp, lt, _ = units.pop()
                units += [(lp, lt, 256), (lp, lt + 256, 256)]
                n_sch = {5: NT - SCH_TAIL + 1, 6: NT - SCH_TAIL + 2}
                tails = {7, 8}
            else:
                n_sch = {len(units) - 3: NT - SCH_TAIL + 1,
                         len(units) - 2: NT - SCH_TAIL + 2}
                tails = {len(units) - 1}
            # Units with hook filler keep PE busy past ACT's exp rate; the
            # late filler-free units need a bigger DVE share, and the tail
            # units drain on both engines at once.
            pending = []  # [(p, t0, pt, w)] up to two units behind
            for i, (p, t0, w) in enumerate(units):
                pt = scores_unit(p, t0, hooks.get(i), n_sch.get(i),
                                 tail=(i in tails), w=w)
                for f in fillers.get(i, []):
                    f()
                if len(pending) == 2:
                    if not skip_attn:
                        attn_unit(*pending.pop(0))
                    else:
                        pending.pop(0)
                pending.append((p, t0, pt, w))
            for args in pending:
                if not skip_attn:
                    attn_unit(*args)

    nc.compile()
    return nc


def _shard_inputs(x, w_Q, w_K, w_V):
    bf = ml_dtypes.bfloat16
    in_maps = []
    for c in range(NCORES):
        b, g = divmod(c, NCORES // B)
        cols = slice(g * GC, (g + 1) * GC)
        in_maps.append({
            "xT": np.ascontiguousarray(np.asarray(x)[b].T).astype(bf),
            "wq": np.ascontiguousarray(np.asarray(w_Q)[:, cols]).astype(bf),
            "wk": np.ascontiguousarray(np.asarray(w_K)[:, cols]).astype(bf),
            "wv": np.ascontiguousarray(np.asarray(w_V)[:, cols]).astype(bf),
        })
    return in_maps


def kernel(x, w_Q, w_K, w_V, _trace=False, _tmpdir=None):
    from concourse.bass_utils import run_bass_kernel_spmd

    global _cached_nc
    if _cached_nc is None:
        _cached_nc = _build_program(T)
    in_maps = _shard_inputs(x, w_Q, w_K, w_V)
    res = run_bass_kernel_spmd(
        _cached_nc, in_maps, list(range(NCORES)),
        trace=_trace, tmpdir=_tmpdir,
    )
    out = np.empty((B, T, E), np.float32)
    for c in range(NCORES):
        b, g = divmod(c, NCORES // B)
        out[b, :, g * GC:(g + 1) * GC] = res.results[c]["out"]
    if _trace:
        return out, res
    return out



# revision 30
# speedup vs baseline: 1.1209x; 1.1209x over previous
"""Multi-head attention (B=2, T=2048, E=1024, H=16) on 8 TRN2 NeuronCores.

Sharding: core c handles batch c//4 and head group c%4 (4 heads of 64 dims
-> 256 columns of w_Q/w_K/w_V and of the output). Pure SPMD, no collectives:
every core runs the same NEFF on its own input shard.

Per-core kernel (all matmul operands bf16, PSUM/softmax math fp32):
  xT [E, T] (host pre-transposed), wq/wk/wv [E, 256]
  1. QT/KT per head-pair p: [128, T] = (w pair-slice)^T @ xT   (PE)
  2. V per s-tile: [128, 4*65] with a ones column per head     (PE + DVE copy)
  3. scores transposed per head: ST[s, t] = K Q^T, two heads packed into
     PE row groups (K=64 each) writing one [128, 1024] PSUM tile
  4. exp via ACT straight from PSUM, scale=1/8 folded into the activation
     affine, bf16 out -> PT
  5. attn: out[t,65] = PT_slice^T @ V_aug accumulated over 16 s-chunks;
     col 64 = softmax denominator (from the ones column)
  6. normalize: DVE reciprocal + per-partition tensor_scalar mul -> fp32 out
"""

import numpy as np
import ml_dtypes

B, T, E, H = 2, 2048, 1024, 16
D = 64          # head dim
HG = 4          # heads per core
GC = HG * D     # 256 output columns per core
NCORES = 8

_cached_nc = None


def _build_program(seq: int = T, reps: int = 1, skip_attn=False, skip_exp=False, _nosplit=True):
    """reps>1 emits the body multiple times in one NEFF (timing only).
    skip_attn/skip_exp build ablation variants for HW phase attribution."""
    import concourse.bacc as bacc
    import concourse.tile as tile
    from concourse import mybir

    bf16 = mybir.dt.bfloat16
    f32 = mybir.dt.float32
    i16 = mybir.dt.int16
    Exp = mybir.ActivationFunctionType.Exp
    Mult = mybir.AluOpType.mult
    Add = mybir.AluOpType.add
    # Schraudolph fast-exp constants (2^x bit trick), with the 1/sqrt(HD)
    # score scale folded into the multiplier like the ACT path's scale=.
    # Scaled by 2^-16 so the rounded result IS the bf16 bit pattern as
    # int16 — one DVE op, no separate bitcast-narrow pass.
    SCH_C1 = (1 << 7) * 1.4426950408889634 * 0.125
    SCH_C2 = (1 << 7) * (127.0 - 0.04367744)
    SCH_TAIL = 11  # exp groups >= this index (per unit) go to DVE

    NT = seq // 128     # s-tiles / t-tiles
    NTC = seq // 512    # 512-wide t-chunks
    KO = E // 128       # contraction chunks for projections

    nc = bacc.Bacc(
        "TRN2", target_bir_lowering=False, debug=False, num_devices=NCORES
    )

    xT_d = nc.dram_tensor("xT", [E, seq], bf16, kind="ExternalInput")
    wq_d = nc.dram_tensor("wq", [E, GC], bf16, kind="ExternalInput")
    wk_d = nc.dram_tensor("wk", [E, GC], bf16, kind="ExternalInput")
    wv_d = nc.dram_tensor("wv", [E, GC], bf16, kind="ExternalInput")
    out_d = nc.dram_tensor("out", [seq, GC], f32, kind="ExternalOutput")

    with tile.TileContext(nc) as tc:
        with (
            tc.tile_pool(name="singles", bufs=1) as singles,
            tc.tile_pool(name="pt", bufs=3) as ptp,
            tc.tile_pool(name="stage", bufs=8) as stagep,
            tc.tile_pool(name="recip", bufs=8) as recipp,
            # PSUM budget (8 banks): scores 3x[128,1024] (6) + attn 1 + proj 1
            tc.tile_pool(name="proj_ps", bufs=1, space="PSUM") as proj_ps,
            tc.tile_pool(name="score_ps", bufs=3, space="PSUM") as score_ps,
            tc.tile_pool(name="attn_ps", bufs=1, space="PSUM") as attn_ps,
        ):
          for _rep in range(reps):
            # ---- load inputs, ramp-ordered: the first projection chains
            # need only the pair-0 weight halves + xT t-chunk 0, so those DMA
            # first and the rest follows behind.
            wq = singles.tile([128, KO, GC], bf16)
            wk = singles.tile([128, KO, GC], bf16)
            wv = singles.tile([128, KO, GC], bf16)
            xT = singles.tile([128, KO, seq], bf16)

            # Weights ride the gpsimd DMA ring so their transfers overlap
            # the xT stream on the sync ring instead of queueing behind it.
            def dma_w(dst, src, p):
                cols = slice(p * 128, (p + 1) * 128)
                nc.gpsimd.dma_start(
                    dst[:, :, cols],
                    src[:, cols].rearrange("(ko p) c -> p ko c", p=128),
                )

            def dma_x(tcq):
                for k in range(KO):
                    nc.sync.dma_start(
                        xT[:, k, tcq * 512:(tcq + 1) * 512],
                        xT_d[k * 128:(k + 1) * 128, tcq * 512:(tcq + 1) * 512],
                    )

            dma_w(wq, wq_d, 0)
            dma_w(wk, wk_d, 0)
            dma_x(0)
            dma_w(wq, wq_d, 1)
            dma_w(wk, wk_d, 1)
            for tcq in range(1, NTC):
                dma_x(tcq)
            nc.gpsimd.dma_start(
                wv[:], wv_d[:].rearrange("(ko p) c -> p ko c", p=128))

            # QT/KT: [128, pair, seq]; partitions 0-63 head 2p, 64-127 head 2p+1
            QT = singles.tile([128, 2, seq], bf16)
            KT = singles.tile([128, 2, seq], bf16)
            # V with ones col per head: [128, s-tile, 4*65]; only the ones
            # columns need the memset — proj_v overwrites the data columns.
            V = singles.tile([128, NT, HG * (D + 1)], bf16)
            nc.vector.memset(
                V[:].rearrange("p st (h c) -> p st h c", h=HG)[:, :, :, D:D + 1],
                1.0,
            )

            def proj_qk(p, w_sb, dst, tcs):
                """Project t-chunks `tcs` of QT or KT for head-pair p."""
                for tcq in tcs:
                    ps = proj_ps.tile([128, 512], f32, tag="proj")
                    for k in range(KO):
                        nc.tensor.matmul(
                            ps[:],
                            lhsT=w_sb[:, k, p * 128:(p + 1) * 128],
                            rhs=xT[:, k, tcq * 512:(tcq + 1) * 512],
                            start=(k == 0),
                            stop=(k == KO - 1),
                        )
                    nc.vector.tensor_copy(
                        out=dst[:, p, tcq * 512:(tcq + 1) * 512], in_=ps[:]
                    )

            def proj_v(tiles):
                for st in tiles:
                    ps = proj_ps.tile([128, 512], f32, tag="proj")
                    for k in range(KO):
                        nc.tensor.matmul(
                            ps[:, :GC],
                            lhsT=xT[:, k, st * 128:(st + 1) * 128],
                            rhs=wv[:, k, :],
                            start=(k == 0),
                            stop=(k == KO - 1),
                        )
                    nc.vector.tensor_copy(
                        out=V[:, st].rearrange("p (h c) -> p h c", h=HG)[:, :, :D],
                        in_=ps[:, :GC].rearrange("p (h c) -> p h c", h=HG),
                    )

            def scores_unit(p, t0, hooks=None, n_sch=None, tail=False,
                            w=512):
                """ST = K Q^T (both heads row-packed) + exp -> PT tile.

                PT layout is flat [128, NT*1024]: one s-tile's two 512-wide
                bank writes form one 2-bank exp group, so attn can chase exp
                output at s-tile granularity. n_sch groups take the DVE
                Schraudolph path: a single tensor_scalar whose rounded int16
                result is the bf16 bit pattern of 2^x (ACT alone can't keep
                up with PE in units that carry no projection filler). The
                last unit alternates engines per group so its exps drain at
                the combined ACT+DVE rate. hooks[st] emits filler just
                before s-tile st."""
                # constant tile shapes regardless of w — half-width units
                # just use the leading columns
                pt = ptp.tile([128, NT * 1024], bf16, tag="pt")
                if n_sch is None:
                    n_sch = NT - SCH_TAIL
                # Spread DVE groups evenly through the unit (clustering them
                # at the tail leaves DVE idle while ACT backlogs, and the
                # score PSUM ring is only 3 groups deep).
                sch_set = {st for st in range(NT)
                           if ((st + 1) * n_sch) // NT > (st * n_sch) // NT}
                for st in range(NT):
                    for f in (hooks or {}).get(st, []):
                        f()
                    sc = score_ps.tile([128, 1024], f32, tag="score")
                    for h in range(2):
                        # heads stay bank-strided (h*512, not h*w): the two
                        # matmuls run concurrently on different PE row
                        # groups, and concurrent writes into one single-port
                        # PSUM bank are a hardware fault.
                        nc.tensor.matmul(
                            sc[:, h * 512:h * 512 + w],
                            lhsT=KT[h * 64:(h + 1) * 64, p,
                                    st * 128:(st + 1) * 128],
                            rhs=QT[h * 64:(h + 1) * 64, p, t0:t0 + w],
                            start=True,
                            stop=True,
                        )
                    if skip_exp:
                        continue
                    dst = pt[:, st * 2 * w:(st + 1) * 2 * w]
                    src = (sc[:] if w == 512 else
                           sc[:].rearrange("p (b c) -> p b c", b=2)[:, :, :w])
                    if (st % 2 == 1) if tail else (st in sch_set):
                        nc.vector.tensor_scalar(
                            dst.bitcast(i16), src, SCH_C1, SCH_C2,
                            Mult, Add,
                        )
                    else:
                        nc.scalar.activation(
                            out=dst, in_=src, func=Exp, scale=0.125,
                        )
                return pt

            def attn_unit(p, t0, pt, w=512):
                """attn = PT^T @ V_aug accumulated over s, then normalize.
                tt-major: both heads' accumulators for one t-tile share a
                PSUM bank (each chain is a closed start..stop group), so
                each t-tile normalizes and DMAs out while the next t-tile's
                chains run — the unit's output drains incrementally instead
                of all at the end."""
                ap2 = attn_ps.tile([128, 2, 2 * (D + 1)], f32, tag="attn")
                for tt in range(w // 128):
                    # alternate halves of the bank: t-tile tt+2's first
                    # (start=True) write only has to wait for tt's normalize
                    # reads, two chains back — not the immediately preceding
                    # ones.
                    ap = ap2[:, tt % 2]
                    stg = stagep.tile([128, 128], f32, tag="stage")
                    for h in range(2):
                        hh = p * 2 + h
                        for st in range(NT):
                            nc.tensor.matmul(
                                ap[:, h * (D + 1):(h + 1) * (D + 1)],
                                lhsT=pt[:, st * 2 * w + h * w + tt * 128:
                                        st * 2 * w + h * w + (tt + 1) * 128],
                                rhs=V[:, st, hh * (D + 1):(hh + 1) * (D + 1)],
                                start=(st == 0),
                                stop=(st == NT - 1),
                            )
                    for h in range(2):
                        r = recipp.tile([128, 1], f32, tag="recip")
                        nc.vector.reciprocal(
                            out=r[:],
                            in_=ap[:, h * (D + 1) + D:h * (D + 1) + D + 1],
                        )
                        nc.vector.tensor_scalar(
                            stg[:, h * D:(h + 1) * D],
                            ap[:, h * (D + 1):h * (D + 1) + D],
                            r[:],
                            None,
                            Mult,
                        )
                    nc.sync.dma_start(
                        out_d[t0 + tt * 128:t0 + (tt + 1) * 128,
                              p * 128:(p + 1) * 128],
                        stg[:],
                    )

            # Program order is semantic order under Tile (WAR/RAW follow it),
            # and it is also the scheduler's priority order. Software-pipeline
            # the softmax: emit scores(u+1) before attn(u) so ACT never
            # starves at a unit boundary; slot filler work (V projection,
            # pair-1 QK, deferred QT-0 chunks) right after the scores that
            # precede its first use.
            # Minimal critical path to the first exp: QT0[tc0], KT0[tc0],
            # then unit-0 scores. All remaining projection work (KT0 tails,
            # QT0 tails, V, pair-1 QK) is spread through the score s-loops
            # as hook filler so PE keeps ACT fed instead of lumping
            # projections between units. attn runs two units behind scores
            # (pt pool bufs >= 3). Everything is emitted before its first
            # program-order use (Tile semantics follow program order).
            proj_qk(0, wq, QT, [0])
            proj_qk(0, wk, KT, [0])
            units = [(p, tcq * 512, 512)
                     for p in range(2) for tcq in range(NTC)]
            qk0 = lambda w, d, tcs: (lambda: proj_qk(0, w, d, tcs))
            qk1 = lambda w, d, tcs: (lambda: proj_qk(1, w, d, tcs))
            pv = lambda ts: (lambda: proj_v(ts))
            if NTC == 4:
                # Each chunk is hooked 2-4 s-tiles before its first use so
                # the PSUM->SBUF copy lands before the dependent ld/mm
                # instead of just-in-time (the copy latency otherwise stalls
                # the score pipeline at every chunk boundary).
                hooks = {
                    0: {2: [qk0(wk, KT, [1])], 5: [qk0(wk, KT, [2])],
                        8: [qk0(wk, KT, [3])], 12: [qk0(wq, QT, [1])]},
                    1: {2: [pv(range(0, 4))], 6: [qk0(wq, QT, [2])],
                        10: [pv(range(4, 8))], 14: [qk0(wq, QT, [3])]},
                    2: {2: [pv(range(8, 12))], 5: [qk1(wk, KT, [0])],
                        8: [pv(range(12, 16))], 11: [qk1(wk, KT, [1])],
                        14: [qk1(wk, KT, [2])]},
                    3: {2: [qk1(wk, KT, [3])], 6: [qk1(wq, QT, [0])],
                        10: [qk1(wq, QT, [1])], 14: [qk1(wq, QT, [2])]},
                    4: {2: [qk1(wq, QT, [3])]},
                }
                fillers = {}
            else:
                hooks = {0: {4 * c: [qk0(wk, KT, [c])] for c in range(1, NTC)}}
                fillers = {0: [qk0(wq, QT, range(1, NTC)),
                               pv(range(NT))]}
                fillers.setdefault(min(1, NTC - 1), []).extend((
                    qk1(wk, KT, range(NTC)),))
                fillers.setdefault(min(2, NTC - 1), []).append(
                    qk1(wq, QT, range(NTC)))
            # The final unit is split into two 256-wide halves so the last
            # exp batch and attn chains are half-sized — the pipeline drain
            # after the last score matmul shortens accordingly.
            if NTC == 4 and not _nosplit:
                lp, lt, _ = units.pop()
                units += [(lp, lt, 256), (lp, lt + 256, 256)]
                n_sch = {5: NT - SCH_TAIL + 1, 6: NT - SCH_TAIL + 2}
                tails = {7, 8}
            else:
                n_sch = {len(units) - 3: NT - SCH_TAIL + 1,
                         len(units) - 2: NT - SCH_TAIL + 2}
                tails = {len(units) - 1}
            # Units with hook filler keep PE busy past ACT's exp rate; the
            # late filler-free units need a bigger DVE share, and the tail
            # units drain on both engines at once.
            pending = []  # [(p, t0, pt, w)] units whose attn hasn't run yet
            nu = len(units)
            for i, (p, t0, w) in enumerate(units):
                pt = scores_unit(p, t0, hooks.get(i), n_sch.get(i),
                                 tail=(i in tails), w=w)
                for f in fillers.get(i, []):
                    f()
                # attn lags scores by 2 units mid-stream (pt/exp slack) but
                # drops to 1 near the end, so only a single attn unit
                # remains after the final score matmul.
                depth = 2 if i < nu - 3 else 1
                while len(pending) >= depth:
                    if not skip_attn:
                        attn_unit(*pending.pop(0))
                    else:
                        pending.pop(0)
                pending.append((p, t0, pt, w))
            for args in pending:
                if not skip_attn:
                    attn_unit(*args)

    nc.compile()
    return nc


def _shard_inputs(x, w_Q, w_K, w_V):
    bf = ml_dtypes.bfloat16
    in_maps = []
    for c in range(NCORES):
        b, g = divmod(c, NCORES // B)
        cols = slice(g * GC, (g + 1) * GC)
        in_maps.append({
            "xT": np.ascontiguousarray(np.asarray(x)[b].T).astype(bf),
            "wq": np.ascontiguousarray(np.asarray(w_Q)[:, cols]).astype(bf),
            "wk": np.ascontiguousarray(np.asarray(w_K)[:, cols]).astype(bf),
            "wv": np.ascontiguousarray(np.asarray(w_V)[:, cols]).astype(bf),
        })
    return in_maps


def kernel(x, w_Q, w_K, w_V, _trace=False, _tmpdir=None):
    from concourse.bass_utils import run_bass_kernel_spmd

    global _cached_nc
    if _cached_nc is None:
        _cached_nc = _build_program(T)
    in_maps = _shard_inputs(x, w_Q, w_K, w_V)
    res = run_bass_kernel_spmd(
        _cached_nc, in_maps, list(range(NCORES)),
        trace=_trace, tmpdir=_tmpdir,
    )
    out = np.empty((B, T, E), np.float32)
    for c in range(NCORES):
        b, g = divmod(c, NCORES // B)
        out[b, :, g * GC:(g + 1) * GC] = res.results[c]["out"]
    if _trace:
        return out, res
    return out

